# revision 37
# baseline (speedup 1.0000x reference)
"""MoE FFN (top-1 routing) on 8 Trainium2 NeuronCores.

Strategy
--------
Host router: logits in fp64 -> argmax matches the fp32 reference exactly
(min top-2 logit gap ~2e-4 >> fp32 matmul noise); tokens are grouped by
expert (stable order), so the grouped-by-expert concatenation IS the
reference output order - no inverse permutation needed.

Device (default impl "v3", hybrid expert-group x ff-split, KERNEL_M=4):
experts are tier-balanced into 8//M groups of M; each group runs on M
cores that split D_FF M ways, partial outputs summed on host.  This
keeps per-core weight DMA at the invariant 16.8MB while x/y shrink to
~4MB (vs 16.8MB for the all-expert ff-parallel v2), eliminating the
shared-HBM saturation that stalled v2's first half, and pads only
sum_k max_g(count of k-th largest expert) token-columns (~1% over
ideal).  Both layers keep weights stationary in the PE array with
tokens as the moving operand: H^T = relu(W1^T X^T + b1),
Y^T_partial = W2^T H^T, fp16 operands / fp32 PSUM (rel err ~5e-4).

Hard-won DMA rules baked in (all measured on hw):
- one partition row = one packet at near-constant cost, so only
  wide-row (>=4KB/row) transfers run near peak - never sub-split
  columns of a [128, N] tile into narrow strips;
- a single queue reaches 300-420GB/s but concurrently active queues
  split the same per-core budget, so ALL bulk traffic rides ONE sync
  queue in exact consumption order;
- HWDGE queues hold ~5 pending transfers and a dma_start beyond that
  blocks the issuing engine's SEQUENCER, so compute engines get at most
  one tiny trigger (b1 on ACT) - everything else on sync, which never
  computes and may block freely;
- the first chains' entire dependency set (x slot0-tile0 + w1 chunk0
  jj0-1) is host-packed adjacently into a single "boot blob" so it
  arrives as ONE wide-row transfer with ONE completion semaphore
  (~1.5us faster first-matmul than two serialized transfers);
- the PE HAM clock ramps over ~5us of sustained activity and drops
  after ~2.5us idle with a painfully slow (~15us) recovery, so dummy
  warm-up matmuls bridge from the preamble to first-data and the
  supply schedule must never let the PE starve mid-stream.
Token tiles are balanced halves (272/264, never 512/24) so LDWEIGHTS
always hides under the moving stream; the final m-tile drains in two
half-tiles to overlap the tail cast+DMA.

KERNEL_IMPL=v2 selects the previous ff-parallel implementation,
KERNEL_IMPL=v1 the simple expert-parallel fallback; KERNEL_M=2 runs the
hybrid with expert pairs (16 ff-tiles/core).
"""

import os
import sys

import numpy as np

sys.path.insert(0, "/opt/trn_rl_repo")

import ml_dtypes  # noqa: E402

D = 1024
E = 8
F = 4096
P = 128
DT = D // P  # 8 d-tiles
FT = F // P  # 32 ff-tiles
MT = D // P  # 8 dout-tiles

BF16 = ml_dtypes.bfloat16

# set by the last kernel() call; test harness reads exec_time_ns from here
last_results = None

_prog_cache = {}


def _ensure_ntff_hook():
    """The agent image's ``antenv`` lacks ``axon_hooks``; install a shim so
    run_bass_kernel_spmd(trace=True) can reach NTFF profiling (degrades to
    no-trace if anything is missing)."""
    try:
        import antenv.axon_hooks  # noqa: F401
        return
    except ImportError:
        pass
    try:
        import types
        import antenv

        mod = types.ModuleType("antenv.axon_hooks")
        _state = {"hook": None}
        mod.set_axon_ntff_profile_hook = lambda h: _state.__setitem__("hook", h)
        mod.get_axon_ntff_profile_hook = lambda: _state["hook"]
        sys.modules["antenv.axon_hooks"] = mod
        antenv.axon_hooks = mod
        try:
            from trn_agent_boot.trn_boot import _ntff_profile_via_ctypes

            mod.set_axon_ntff_profile_hook(
                _ntff_profile_via_ctypes("/opt/axon/libaxon_pjrt.so")
            )
        except Exception:
            pass
    except Exception:
        pass


def _tok_tiles(C):
    """Split C tokens into moving-operand tiles of <=512 (PSUM bank limit)."""
    tiles = []
    t0 = 0
    while t0 < C:
        tn = min(512, C - t0)
        tiles.append((t0, tn))
        t0 += tn
    return tiles


def _build(C, compute_dt_name):
    import concourse.mybir as mybir
    from concourse import bacc
    from concourse.tile import TileContext

    cdt = getattr(mybir.dt, compute_dt_name)
    f32 = mybir.dt.float32
    AF = mybir.ActivationFunctionType

    tok = _tok_tiles(C)
    nc = bacc.Bacc(
        "TRN2",
        target_bir_lowering=False,
        debug=False,
        enable_asserts=False,
        num_devices=E,
    )

    xt_d = nc.declare_dram_parameter("xt", [P, DT * C], cdt, isOutput=False)
    w1_d = nc.declare_dram_parameter("w1", [DT, P, 4 * DT * P], cdt, isOutput=False)
    w2_d = nc.declare_dram_parameter("w2", [MT, P, FT * P], cdt, isOutput=False)
    b1_d = nc.declare_dram_parameter("b1", [P, FT], f32, isOutput=False)
    b2_d = nc.declare_dram_parameter("b2", [P, MT], f32, isOutput=False)
    yt_d = nc.declare_dram_parameter("yt", [MT, P, C], f32, isOutput=True)

    with TileContext(nc) as tc:
        with (
            tc.tile_pool(name="const", bufs=1) as constp,
            tc.tile_pool(name="xp", bufs=1) as xp,
            tc.tile_pool(name="w1p", bufs=3) as w1p,
            tc.tile_pool(name="w2p", bufs=3) as w2p,
            tc.tile_pool(name="hp", bufs=1) as hp,
            tc.tile_pool(name="yp", bufs=2) as yp,
            tc.tile_pool(name="ps1", space="PSUM", bufs=2) as ps1,
            tc.tile_pool(name="ps2", space="PSUM", bufs=2) as ps2,
        ):
            x_sb = xp.tile([P, DT * C], cdt, tag="x", name="x_sb")
            nc.sync.dma_start(x_sb[:], xt_d[:])
            b1_sb = constp.tile([P, FT], f32, tag="b1", name="b1_sb")
            nc.sync.dma_start(b1_sb[:], b1_d[:])
            b2_sb = constp.tile([P, MT], f32, tag="b2", name="b2_sb")
            nc.sync.dma_start(b2_sb[:], b2_d[:])

            h_tiles = [
                hp.tile([P, C], cdt, tag=f"h{j}", name=f"h{j}") for j in range(FT)
            ]

            # ---- layer 1: H^T[j] = relu(W1^T X^T + b1), j = ff tile ----
            for jb in range(DT):  # 8 chunks of 4 ff-tiles (1MB each)
                w1_sb = w1p.tile([P, 4 * DT * P], cdt, tag="w1c", bufs=3,
                                 name=f"w1c{jb}")
                nc.sync.dma_start(w1_sb[:], w1_d[jb])
                for jj in range(4):
                    j = jb * 4 + jj
                    pss = [
                        ps1.tile([P, tn], f32, tag=f"psA{ti}", bufs=2,
                                 name=f"ps_{j}_{ti}")
                        for ti, (t0, tn) in enumerate(tok)
                    ]
                    for i in range(DT):
                        lhsT = w1_sb[:, (jj * DT + i) * P:(jj * DT + i + 1) * P]
                        for ti, (t0, tn) in enumerate(tok):
                            nc.tensor.matmul(
                                pss[ti][:],
                                lhsT,
                                x_sb[:, i * C + t0:i * C + t0 + tn],
                                start=(i == 0),
                                stop=(i == DT - 1),
                            )
                    for ti, (t0, tn) in enumerate(tok):
                        nc.scalar.activation(
                            h_tiles[j][:, t0:t0 + tn],
                            pss[ti][:],
                            AF.Relu,
                            bias=b1_sb[:, j:j + 1],
                        )

            # ---- layer 2: Y^T[m] = W2^T H^T + b2, m = dout tile ----
            for m in range(MT):
                w2_sb = w2p.tile([P, FT * P], cdt, tag="w2c", bufs=3,
                                 name=f"w2c{m}")
                nc.sync.dma_start(w2_sb[:], w2_d[m])
                y_sb = yp.tile([P, C], f32, tag="y", bufs=2, name=f"y{m}")
                pss = [
                    ps2.tile([P, tn], f32, tag=f"psB{ti}", bufs=2,
                             name=f"psy_{m}_{ti}")
                    for ti, (t0, tn) in enumerate(tok)
                ]
                for j in range(FT):
                    lhsT = w2_sb[:, j * P:(j + 1) * P]
                    for ti, (t0, tn) in enumerate(tok):
                        nc.tensor.matmul(
                            pss[ti][:],
                            lhsT,
                            h_tiles[j][:, t0:t0 + tn],
                            start=(j == 0),
                            stop=(j == FT - 1),
                        )
                for ti, (t0, tn) in enumerate(tok):
                    nc.scalar.activation(
                        y_sb[:, t0:t0 + tn],
                        pss[ti][:],
                        AF.Identity,
                        bias=b2_sb[:, m:m + 1],
                    )
                nc.sync.dma_start(yt_d[m], y_sb[:])

    nc.compile()
    return nc


_BUILDER_SRC = r'''"""Device-program builder for the MoE kernel.

This file is written by kernel.py to a content-addressed path under /tmp
and imported from there, so the Bass-captured debug info (source path,
line numbers) -- and therefore the generated BIR bytes and the neuronx
compile-cache key -- are identical no matter where kernel.py itself
lives.
"""

import sys

if "/opt/trn_rl_repo" not in sys.path:
    sys.path.insert(0, "/opt/trn_rl_repo")

D = 1024
E = 8
F = 4096
P = 128
DT = D // P
FT = F // P
MT = D // P


def _tok_tiles(C):
    tiles = []
    t0 = 0
    while t0 < C:
        tn = min(512, C - t0)
        tiles.append((t0, tn))
        t0 += tn
    return tiles


def build_v2(ces):
    """ff-parallel: every core runs ALL experts, but only 4 of the 32 ff
    tiles (its quarter of D_FF, baked into its weight data by the host).
    Partial outputs (fp16) are summed on the host. PE streams exactly
    sum(C_e) columns per (i|m)-tile instead of 8*max(C_e)."""
    import concourse.mybir as mybir
    from concourse import bacc
    from concourse.tile import TileContext

    cdt = mybir.dt.float16
    f32 = mybir.dt.float32
    f16 = mybir.dt.float16
    AF = mybir.ActivationFunctionType

    CT = sum(ces)
    xbase = [sum(ces[:e]) for e in range(E)]
    toks = [_tok_tiles(c) for c in ces]

    nc = bacc.Bacc(
        "TRN2",
        target_bir_lowering=False,
        debug=False,
        enable_asserts=False,
        num_devices=E,
    )

    xt_d = nc.declare_dram_parameter("xt", [P, DT * CT], cdt, isOutput=False)
    w1_d = nc.declare_dram_parameter("w1", [E, P, 4 * DT * P], cdt, isOutput=False)
    w2_d = nc.declare_dram_parameter("w2", [E, P, 4 * MT * P], cdt, isOutput=False)
    b1_d = nc.declare_dram_parameter("b1", [P, E * 4], f32, isOutput=False)
    y_ds = [
        nc.declare_dram_parameter(f"y{e}", [P, MT * ces[e]], f16, isOutput=True)
        for e in range(E)
    ]

    with TileContext(nc) as tc:
        with (
            tc.tile_pool(name="const", bufs=1) as constp,
            tc.tile_pool(name="xp", bufs=1) as xp,
            tc.tile_pool(name="w1p", bufs=4) as w1p,
            tc.tile_pool(name="w2p", bufs=4) as w2p,
            tc.tile_pool(name="hp", bufs=2) as hp,
            tc.tile_pool(name="yp", bufs=2) as yp,
            tc.tile_pool(name="ps1", space="PSUM", bufs=2) as ps1,
            tc.tile_pool(name="ps2", space="PSUM", bufs=2) as ps2,
        ):
            x_sb = xp.tile([P, DT * CT], cdt, tag="x", name="x_sb")
            w1_sbs = {}
            h_all = {}

            def dma_x(e, nsplit, first_engine=None):
                xb = DT * xbase[e]
                n = DT * ces[e]
                step = (n + nsplit - 1) // nsplit
                for s in range(0, n, step):
                    w = min(step, n - s)
                    eng = first_engine if (s == 0 and first_engine) else nc.sync
                    eng.dma_start(
                        x_sb[:, xb + s:xb + s + w], xt_d[:, xb + s:xb + s + w]
                    )

            def dma_w1(e, nsplit=1, first_engine=None):
                w1_sb = w1p.tile([P, 4 * DT * P], cdt, tag="w1c", name=f"w1c{e}")
                w1_sbs[e] = w1_sb
                step = 4 * DT * P // nsplit
                for s in range(0, 4 * DT * P, step):
                    eng = first_engine if (s == 0 and first_engine) else nc.scalar
                    eng.dma_start(
                        w1_sb[:, s:s + step], w1_d[e, :, s:s + step]
                    )

            def emit_l1(e):
                Ce = ces[e]
                xb = DT * xbase[e]
                tok = toks[e]
                w1_sb = w1_sbs[e]
                h_all[e] = []
                for jj in range(4):
                    h = hp.tile([P, Ce], cdt, tag=f"h{jj}", bufs=3,
                                name=f"h{e}_{jj}")
                    h_all[e].append(h)
                    pss = [
                        ps1.tile([P, tn], f32, tag=f"psA{ti}",
                                 name=f"ps_{e}_{jj}_{ti}")
                        for ti, (t0, tn) in enumerate(tok)
                    ]
                    for i in range(DT):
                        lhsT = w1_sb[:, (jj * DT + i) * P:(jj * DT + i + 1) * P]
                        for ti, (t0, tn) in enumerate(tok):
                            nc.tensor.matmul(
                                pss[ti][:],
                                lhsT,
                                x_sb[:, xb + i * Ce + t0:xb + i * Ce + t0 + tn],
                                start=(i == 0),
                                stop=(i == DT - 1),
                            )
                    for ti, (t0, tn) in enumerate(tok):
                        nc.scalar.activation(
                            h[:, t0:t0 + tn],
                            pss[ti][:],
                            AF.Relu,
                            bias=b1_sb[:, e * 4 + jj:e * 4 + jj + 1],
                        )

            w2_sbs = {}

            def dma_w2(e):
                w2_sb = w2p.tile([P, 4 * MT * P], cdt, tag="w2c", name=f"w2c{e}")
                w2_sbs[e] = w2_sb
                nc.scalar.dma_start(w2_sb[:], w2_d[e])

            def emit_l2(e):
                Ce = ces[e]
                tok = toks[e]
                w2_sb = w2_sbs.pop(e)
                y_sb = yp.tile([P, MT * Ce], f16, tag="y", name=f"y{e}")
                for m in range(MT):
                    pss = [
                        ps2.tile([P, tn], f32, tag=f"psB{ti}",
                                 name=f"psy_{e}_{m}_{ti}")
                        for ti, (t0, tn) in enumerate(tok)
                    ]
                    for jj in range(4):
                        lhsT = w2_sb[:, (jj * MT + m) * P:(jj * MT + m + 1) * P]
                        for ti, (t0, tn) in enumerate(tok):
                            nc.tensor.matmul(
                                pss[ti][:],
                                lhsT,
                                h_all[e][jj][:, t0:t0 + tn],
                                start=(jj == 0),
                                stop=(jj == 3),
                            )
                    for ti, (t0, tn) in enumerate(tok):
                        nc.vector.tensor_copy(
                            y_sb[:, m * Ce + t0:m * Ce + t0 + tn],
                            pss[ti][:],
                        )
                    if e == E - 1:
                        nc.sync.dma_start(
                            y_ds[e][:, m * Ce:(m + 1) * Ce],
                            y_sb[:, m * Ce:(m + 1) * Ce],
                        )
                if e != E - 1:
                    nc.sync.dma_start(y_ds[e][:], y_sb[:])
                del h_all[e]

            # startup: x0 per-i on SP lane, w1c0 per-jj on ACT lane, so the
            # first matmul starts as soon as x0_i0 + w1c0_jj0 land.
            dma_x(0, 4)
            dma_w1(0, nsplit=4)
            b1_sb = constp.tile([P, E * 4], f32, tag="b1", name="b1_sb")
            nc.scalar.dma_start(b1_sb[:], b1_d[:])

            # PE warm-up: dummy matmuls on (uninitialized) scratch while the
            # first input DMAs are in flight, so HAM is at K=8/8 when real
            # work starts. The psum result is never read.
            warm_w = constp.tile([P, P], cdt, tag="warmw", name="warm_w")
            warm_x = constp.tile([P, 256], cdt, tag="warmx", name="warm_x")
            nc.vector.memset(warm_w[:], 0.0)
            nc.vector.memset(warm_x[:], 0.0)
            warm_ps = ps2.tile([P, 256], f32, tag="psB0", name="warm_ps")
            for w in range(40):
                nc.tensor.matmul(
                    warm_ps[:], warm_w[:], warm_x[:],
                    start=(w == 0), stop=(w == 39),
                )
            # L1 runs one expert ahead of L2: L2(e-1) is ready-to-run PE work
            # that absorbs any DMA lateness in L1(e)'s inputs.
            emit_l1(0)
            dma_x(1, 2)
            dma_w1(1)
            dma_w2(0)
            emit_l1(1)
            for e in range(2, E):
                emit_l2(e - 2)
                dma_x(e, 2)
                dma_w1(e)
                dma_w2(e - 1)
                emit_l1(e)
            dma_w2(E - 1)
            emit_l2(E - 2)
            emit_l2(E - 1)

    nc.compile()
    return nc


def build_moe(cs, jt):
    """Hybrid expert-group x ff-split: 8//NS groups of NS experts, each
    group on NS cores that split D_FF NS ways. Per-core DMA is ~21-25MB
    (weights 16.8MB invariant + x/y) vs 33.8MB for the all-expert v2,
    which avoids shared-HBM saturation, while compute stays near-balanced:
    every core streams sum(cs) padded token-columns through jt ff-tiles x
    8 d-tiles x 2 layers.

    SPMD: all 8 cores run this one program; the (group, ff-part) identity
    lives entirely in the host-packed weight/x data. Token tiles are
    BALANCED halves (272/264 instead of 512/24) so no matmul is ever too
    narrow to hide LDWEIGHTS under the moving stream."""
    import concourse.mybir as mybir
    from concourse import bacc
    from concourse.tile import TileContext

    f16 = mybir.dt.float16
    f32 = mybir.dt.float32
    AF = mybir.ActivationFunctionType

    cs = list(cs)
    NS = len(cs)       # expert slots per group (= cores per group)
    JT = jt            # ff tiles per core (32 // NS)
    NC1 = JT // 4      # w1 chunks per slot (4 jj-tiles each)
    CT = sum(cs)

    def ttiles(C):
        if C <= 512:
            return [(0, C)]
        h = (C // 2 + 7) // 8 * 8
        return [(0, h), (h, C - h)]

    toks = [ttiles(c) for c in cs]
    xoff = [DT * sum(cs[:s]) for s in range(NS)]

    nc = bacc.Bacc(
        "TRN2",
        target_bir_lowering=False,
        debug=False,
        enable_asserts=False,
        num_devices=E,
    )

    xt_d = nc.declare_dram_parameter("xt", [P, DT * CT], f16, isOutput=False)
    tn0 = toks[0][0][1]
    BOOT = DT * tn0 + 2 * DT * P  # x slot0-tile0 + w1 chunk0 jj0-1
    boot_d = nc.declare_dram_parameter("boot", [P, BOOT], f16, isOutput=False)
    w1_d = nc.declare_dram_parameter(
        "w1", [NS * NC1, P, 4 * DT * P], f16, isOutput=False
    )
    w2_d = nc.declare_dram_parameter(
        "w2", [NS * 4, P, 2 * JT * P], f16, isOutput=False
    )
    b1_d = nc.declare_dram_parameter("b1", [P, NS * JT], f32, isOutput=False)
    y_ds = [
        nc.declare_dram_parameter(f"y{s}", [P, MT * cs[s]], f16, isOutput=True)
        for s in range(NS)
    ]

    with TileContext(nc) as tc:
        with (
            tc.tile_pool(name="const", bufs=1) as constp,
            tc.tile_pool(name="xp", bufs=1) as xp,
            tc.tile_pool(name="w1p", bufs=2) as w1p,
            tc.tile_pool(name="w2p", bufs=2) as w2p,
            tc.tile_pool(name="hp", bufs=2) as hp,
            tc.tile_pool(name="yp", bufs=1) as yp,
            tc.tile_pool(name="ps1", space="PSUM", bufs=2) as ps1,
            tc.tile_pool(name="ps2", space="PSUM", bufs=2) as ps2,
        ):
            x_sb = xp.tile([P, DT * CT], f16, tag="x", name="x_sb")
            b1_sb = constp.tile([P, NS * JT], f32, tag="b1", name="b1_sb")

            # DMA plan (all limits measured on hw):
            # - one partition row = one packet at ~constant cost, so only
            #   wide-row transfers (>=4KB/row) run near peak; never
            #   sub-split columns below that.
            # - a lone queue reaches ~300-420GB/s; two+ queues split the
            #   same per-core budget, so everything goes on ONE queue in
            #   exact consumption order.
            # - HWDGE queue depth is ~5 pending transfers and a dma_start
            #   beyond that BLOCKS the issuing engine's sequencer, so the
            #   queue lives on sync (never computes); scalar only gets the
            #   single tiny b1 trigger.
            nc.scalar.dma_start(b1_sb[:], b1_d[:])
            w1_sbs = {}
            for s in range(NS):
                for c in range(NC1):
                    w1_sbs[(s, c)] = w1p.tile(
                        [P, 4 * DT * P], f16, tag=f"w1c{c}", name=f"w1_{s}_{c}"
                    )
            w2_sbs = {}
            for s in range(NS):
                for c in range(4):
                    w2_sbs[(s, c)] = w2p.tile(
                        [P, 2 * JT * P], f16, tag=f"w2c{c}",
                        name=f"w2_{s}_{c}"
                    )

            def dma_x(s):
                for (t0, tn) in toks[s]:
                    a = xoff[s] + DT * t0
                    nc.sync.dma_start(
                        x_sb[:, a:a + DT * tn], xt_d[:, a:a + DT * tn]
                    )

            def dma_w1(s, c):
                nc.sync.dma_start(w1_sbs[(s, c)][:], w1_d[s * NC1 + c])

            def dma_w2(s):
                for c in range(4):
                    nc.sync.dma_start(w2_sbs[(s, c)][:], w2_d[s * 4 + c])

            # slot0 startup: the "boot blob" = x slot0-tile0 + w1 chunk0
            # jj0-1 host-packed adjacently so the first chains' ENTIRE
            # dependency set arrives as ONE wide-row transfer with ONE
            # completion semaphore (saves a serialized transfer + sem on
            # the critical path); then chunk0's jj2-3 half, x tile1, the
            # rest of slot0; later slots stream in consumption order with
            # w2(s) after x/w1(s+1) (L2(s) runs after L1(s+1)).
            boot_sb = constp.tile([P, BOOT], f16, tag="boot", name="boot_sb")
            nc.sync.dma_start(boot_sb[:], boot_d[:])
            nc.sync.dma_start(
                w1_sbs[(0, 0)][:, 2 * DT * P:4 * DT * P],
                w1_d[0, :, 2 * DT * P:4 * DT * P],
            )
            for (t0, tn) in toks[0][1:]:
                a = xoff[0] + DT * t0
                nc.sync.dma_start(
                    x_sb[:, a:a + DT * tn], xt_d[:, a:a + DT * tn]
                )
            for c in range(1, NC1):
                dma_w1(0, c)
            for s in range(1, NS):
                dma_x(s)
                for c in range(NC1):
                    dma_w1(s, c)
                dma_w2(s - 1)
            dma_w2(NS - 1)

            # ---- PE warm-up: ramps the HAM clock while startup DMAs fly.
            warm_w = constp.tile([P, P], f16, tag="warmw", name="warm_w")
            warm_x = constp.tile([P, 256], f16, tag="warmx", name="warm_x")
            nc.vector.memset(warm_w[:], 0.0)
            nc.vector.memset(warm_x[:], 0.0)
            warm_ps = ps2.tile([P, 256], f32, tag="psB0", name="warm_ps")
            for w in range(24):
                nc.tensor.matmul(
                    warm_ps[:], warm_w[:], warm_x[:],
                    start=(w == 0), stop=(w == 23),
                )

            h_all = {}

            def l1_chain(s, jj, ti):
                C = cs[s]
                base = xoff[s]
                t0, tn = toks[s][ti]
                w1_sb = w1_sbs[(s, jj // 4)]
                jl = jj % 4
                ps = ps1.tile([P, tn], f32, tag=f"psA{ti}",
                              name=f"ps_{s}_{jj}_{ti}")
                tb = base + DT * t0
                # slot0-tile0 x and slot0 jj0-1 weights live in the boot blob
                xv = boot_sb if (s == 0 and ti == 0) else x_sb
                xb = 0 if (s == 0 and ti == 0) else tb
                if s == 0 and jj < 2:
                    wv, wb = boot_sb, DT * tn0
                else:
                    wv, wb = w1_sb, 0
                for i in range(DT):
                    nc.tensor.matmul(
                        ps[:],
                        wv[:, wb + (jl * DT + i) * P:wb + (jl * DT + i + 1) * P],
                        xv[:, xb + i * tn:xb + (i + 1) * tn],
                        start=(i == 0),
                        stop=(i == DT - 1),
                    )
                nc.scalar.activation(
                    h_all[(s, jj)][:, t0:t0 + tn],
                    ps[:],
                    AF.Relu,
                    bias=b1_sb[:, s * JT + jj:s * JT + jj + 1],
                )

            def emit_l1(s):
                C = cs[s]
                for jj in range(JT):
                    h_all[(s, jj)] = hp.tile([P, C], f16, tag=f"h{jj}",
                                             name=f"h{s}_{jj}")
                ntile = len(toks[s])
                if s == 0 and ntile == 2:
                    # chunk-0 runs tile0 for jj 0..3 first: the first chains
                    # need only x tile0 + the per-jj w1 pieces, all of which
                    # land within ~12us; x tile1 arrives during this pass
                    for ti in range(2):
                        for jj in range(4):
                            l1_chain(s, jj, ti)
                    rest = range(4, JT)
                else:
                    rest = range(JT)
                for jj in rest:
                    for ti in range(ntile):
                        l1_chain(s, jj, ti)

            def emit_l2(s):
                C = cs[s]
                y_sb = yp.tile([P, MT * C], f16, tag=f"y{s % 2}", name=f"y{s}")
                for m in range(MT):
                    w2_sb = w2_sbs[(s, m // 2)]
                    ml = m % 2
                    last = (s == NS - 1 and m == MT - 1)
                    # the final m-tile runs as two half-token chains so its
                    # first half's cast+DMA overlaps the second half's chain
                    tiles = toks[s]
                    if last and len(tiles) == 1 and C >= 64:
                        h2 = (C // 2 + 7) // 8 * 8
                        tiles = [(0, h2), (h2, C - h2)]
                    for ti, (t0, tn) in enumerate(tiles):
                        ps = ps2.tile([P, tn], f32, tag=f"psB{ti}",
                                      name=f"psy_{s}_{m}_{ti}")
                        for j in range(JT):
                            nc.tensor.matmul(
                                ps[:],
                                w2_sb[:, ((ml * JT + j) * P):((ml * JT + j + 1) * P)],
                                h_all[(s, j)][:, t0:t0 + tn],
                                start=(j == 0),
                                stop=(j == JT - 1),
                            )
                        nc.vector.tensor_copy(
                            y_sb[:, m * C + t0:m * C + t0 + tn], ps[:]
                        )
                        if last:
                            nc.sync.dma_start(
                                y_ds[s][:, m * C + t0:m * C + t0 + tn],
                                y_sb[:, m * C + t0:m * C + t0 + tn],
                            )
                    if not last:
                        nc.sync.dma_start(
                            y_ds[s][:, m * C:(m + 1) * C],
                            y_sb[:, m * C:(m + 1) * C],
                        )

            # one-slot-ahead pipeline: L1 runs one slot ahead of L2 so L2's
            # long-resident inputs absorb any DMA lateness
            emit_l1(0)
            emit_l1(1)
            for s in range(2, NS):
                emit_l2(s - 2)
                emit_l1(s)
            emit_l2(NS - 2)
            emit_l2(NS - 1)

    nc.compile()
    return nc


def build_v2_into(ces, out):
    # thread entrypoint: keeps caller frames (kernel.py, driver) out of the
    # Bass-captured tracebacks so the BIR bytes are fully location-independent
    try:
        out["nc"] = build_v2(ces)
    except BaseException as exc:  # noqa: BLE001
        out["exc"] = exc


def build_moe_into(cs, jt, out):
    try:
        out["nc"] = build_moe(cs, jt)
    except BaseException as exc:  # noqa: BLE001
        out["exc"] = exc
'''


def _builder_mod():
    """Import the builder via a content-addressed module under /tmp so the
    generated BIR (and hence the neuron compile-cache key) is independent
    of where this file lives."""
    import hashlib
    import importlib.util

    h = hashlib.md5(_BUILDER_SRC.encode()).hexdigest()[:12]
    modname = f"_moe_builder_{h}"
    if modname not in sys.modules:
        path = f"/tmp/_moe_builder_{h}.py"
        try:
            if not (os.path.exists(path)
                    and open(path).read() == _BUILDER_SRC):
                tmp = f"{path}.{os.getpid()}.tmp"
                with open(tmp, "w") as f:
                    f.write(_BUILDER_SRC)
                os.replace(tmp, path)
        except OSError:
            import tempfile

            path = os.path.join(tempfile.mkdtemp(), f"{modname}.py")
            with open(path, "w") as f:
                f.write(_BUILDER_SRC)
        spec = importlib.util.spec_from_file_location(modname, path)
        mod = importlib.util.module_from_spec(spec)
        sys.modules[modname] = mod
        spec.loader.exec_module(mod)
    return sys.modules[modname]


def _build_in_thread(fn_name, args):
    # thread entrypoint keeps caller frames (kernel.py, driver) out of the
    # Bass-captured tracebacks so the BIR bytes are location-independent
    import threading

    mod = _builder_mod()
    out = {}
    t = threading.Thread(target=getattr(mod, fn_name), args=(*args, out))
    t.start()
    t.join()
    if "exc" in out:
        raise out["exc"]
    return out["nc"]


def _build_v2(ces):
    return _build_in_thread("build_v2_into", (ces,))


def _build_moe(cs, jt):
    return _build_in_thread("build_moe_into", (list(cs), jt))


def _run_with_retry(run_fn, nc, in_maps, tmpdir, attempts=4):
    """Transient NRT/device errors (e.g. NRT_EXEC_UNIT_UNRECOVERABLE right
    after another process released the cores) have been observed; retry with
    growing backoff, resetting the jax backend in between (the failed PJRT
    client state does not recover on its own)."""
    import time

    last_exc = None
    for a in range(attempts):
        try:
            return run_fn(nc, in_maps, core_ids=list(range(E)), tmpdir=tmpdir)
        except Exception as exc:  # noqa: BLE001
            last_exc = exc
            time.sleep(5.0 * (a + 1))
            try:
                import jax

                jax.clear_backends()
            except Exception:
                pass
    raise last_exc


def kernel(x, Wg, bg, W1, b1, W2, b2, k):
    global last_results
    _ensure_ntff_hook()
    from concourse.bass_utils import run_bass_kernel_spmd

    compute_dt = os.environ.get("KERNEL_COMPUTE_DT", "bfloat16")
    np_cdt = BF16 if compute_dt == "bfloat16" else np.float32

    impl = os.environ.get("KERNEL_IMPL", "v3")

    x = np.asarray(x)
    B, S, _ = x.shape
    N = B * S
    x_flat = np.ascontiguousarray(x.reshape(N, D)).astype(np.float32)

    # ---- host router (exact vs fp32 reference; see module docstring) ----
    logits = x_flat.astype(np.float64) @ np.asarray(Wg).astype(np.float64)
    logits += np.asarray(bg).astype(np.float64)
    assign = np.argmax(logits, axis=-1)

    idx_per_e = [np.flatnonzero(assign == e) for e in range(E)]
    counts = [len(ix) for ix in idx_per_e]

    W1 = np.asarray(W1, dtype=np.float32)
    W2 = np.asarray(W2, dtype=np.float32)
    b1 = np.asarray(b1, dtype=np.float32)
    b2 = np.asarray(b2, dtype=np.float32)

    tmpdir = os.environ.get("KERNEL_TMPDIR")

    if impl == "v3":
        # ---- hybrid expert-group x ff-split: ngroups groups of M experts,
        # each group on M cores splitting D_FF M ways (partials summed on
        # host).  M=4 minimizes padded columns (2072 vs 2x1048) ----
        M = int(os.environ.get("KERNEL_M", "4"))
        ngroups = E // M
        JT = FT // M  # ff tiles per core
        NC1 = JT // 4
        FS = F // M   # ff columns per core
        order = list(np.argsort([-c for c in counts], kind="stable"))
        # position k of every group takes one expert from the k-th tier of
        # sorted counts (tiers alternate direction to balance group sums)
        groups = [[0] * M for _ in range(ngroups)]
        cs = []
        for k in range(M):
            tier = order[k * ngroups:(k + 1) * ngroups]
            if k % 2:
                tier = tier[::-1]
            for g in range(ngroups):
                groups[g][k] = tier[g]
            # pad to even only: DMA rows are byte-granular, PSUM/matmul
            # take any width, so multiple-of-8 padding just wastes columns
            cs.append(max(8, (max(counts[e] for e in tier) + 1) // 2 * 2))

        def ttiles(C):
            if C <= 512:
                return [(0, C)]
            h = (C // 2 + 7) // 8 * 8
            return [(0, h), (h, C - h)]

        # x per group: token-tile-major [P, (slot, tile, i, t)] fp16
        xts = []
        for g in range(ngroups):
            blocks = []
            for s in range(M):
                e = groups[g][s]
                C = cs[s]
                xp_ = np.zeros((C, D), np.float32)
                xp_[:counts[e]] = x_flat[idx_per_e[e]]
                xT = xp_.T  # [D, C]
                for (t0, tn) in ttiles(C):
                    blocks.append(
                        xT[:, t0:t0 + tn].reshape(DT, P, tn)
                        .transpose(1, 0, 2).reshape(P, DT * tn)
                    )
            xts.append(
                np.ascontiguousarray(np.concatenate(blocks, axis=1))
                .astype(np.float16)
            )

        in_maps = []
        for kcore in range(E):
            g, hpart = kcore // M, kcore % M
            w1c = np.empty((M * NC1, P, 4 * DT * P), np.float16)
            w2c = np.empty((M * 4, P, 2 * JT * P), np.float16)
            b1c = np.empty((P, M * JT), np.float32)
            for s in range(M):
                e = groups[g][s]
                # W1 part: [D, FS] -> chunks of 4 jj-tiles, layout
                # [p, ((jl*DT+i)*P)+cc]
                W1h = W1[e][:, hpart * FS:(hpart + 1) * FS].reshape(
                    DT, P, JT, P
                )
                for c in range(NC1):
                    w1c[NC1 * s + c] = (
                        W1h[:, :, 4 * c:4 * c + 4, :]
                        .transpose(1, 2, 0, 3).reshape(P, 4 * DT * P)
                    )
                # W2 part: [FS, D] -> chunks of 2 m-tiles, layout
                # [p, ((ml*JT+j)*P)+cc]
                W2h = W2[e][hpart * FS:(hpart + 1) * FS, :].reshape(
                    JT, P, MT, P
                )
                for c in range(4):
                    w2c[4 * s + c] = (
                        W2h[:, :, 2 * c:2 * c + 2, :]
                        .transpose(1, 2, 0, 3).reshape(P, 2 * JT * P)
                    )
                b1c[:, s * JT:(s + 1) * JT] = (
                    b1[e][hpart * FS:(hpart + 1) * FS].reshape(JT, P).T
                )
            tn0 = ttiles(cs[0])[0][1]
            boot = np.ascontiguousarray(np.concatenate(
                [xts[g][:, :DT * tn0], w1c[0][:, :2 * DT * P]], axis=1
            ))
            in_maps.append({
                "xt": xts[g],
                "boot": boot,
                "w1": np.ascontiguousarray(w1c),
                "w2": np.ascontiguousarray(w2c),
                "b1": b1c,
            })

        key = ("moe", M, tuple(cs))
        if key not in _prog_cache:
            _prog_cache[key] = _build_moe(cs, JT)
        nc = _prog_cache[key]

        last_results = _run_with_retry(
            run_bass_kernel_spmd, nc, in_maps, tmpdir
        )

        loc = {}  # expert -> (group, slot)
        for g in range(ngroups):
            for s in range(M):
                loc[groups[g][s]] = (g, s)
        out = np.empty((N, D), np.float32)
        pos = 0
        for e in range(E):
            g, s = loc[e]
            C = cs[s]
            cnt = counts[e]
            acc = last_results.results[M * g][f"y{s}"].astype(np.float32)
            for hpart in range(1, M):
                acc += last_results.results[M * g + hpart][f"y{s}"]
            ye = acc.reshape(P, MT, C).transpose(1, 0, 2).reshape(D, C).T[:cnt]
            out[pos:pos + cnt] = ye + b2[e]
            pos += cnt
        return out.reshape(B, S, D)

    if impl == "v2":
        # slot order: largest expert first (more PE work early to cover the
        # DMA supply ramp; smallest expert last shortens the drain tail)
        perm = list(np.argsort([-c for c in counts], kind="stable"))
        ces = [max(8, (counts[p] + 7) // 8 * 8) for p in perm]
        CT = sum(ces)
        xbase = [sum(ces[:s]) for s in range(E)]

        # shared x: per-slot blocks of [P, DT*Ce]
        xt = np.zeros((P, DT * CT), np.float32)
        for s in range(E):
            e = perm[s]
            xp_ = np.zeros((ces[s], D), np.float32)
            xp_[:counts[e]] = x_flat[idx_per_e[e]]
            xt[:, DT * xbase[s]:DT * xbase[s] + DT * ces[s]] = (
                xp_.T.reshape(DT, P, ces[s]).transpose(1, 0, 2)
                .reshape(P, DT * ces[s])
            )
        xt = np.ascontiguousarray(xt).astype(np.float16)

        W1r = W1[perm].reshape(E, DT, P, FT, P)
        W2r = W2[perm].reshape(E, FT, P, MT, P)
        b1r = b1[perm].reshape(E, FT, P)
        in_maps = []
        for kcore in range(E):
            js = slice(4 * kcore, 4 * kcore + 4)
            w1c = np.ascontiguousarray(
                W1r[:, :, :, js, :].transpose(0, 2, 3, 1, 4)
                .reshape(E, P, 4 * DT * P)
            ).astype(np.float16)
            w2c = np.ascontiguousarray(
                W2r[:, js, :, :, :].transpose(0, 2, 1, 3, 4)
                .reshape(E, P, 4 * MT * P)
            ).astype(np.float16)
            b1c = np.ascontiguousarray(
                b1r[:, js, :].transpose(2, 0, 1).reshape(P, E * 4)
            )
            in_maps.append({"xt": xt, "w1": w1c, "w2": w2c, "b1": b1c})

        key = ("v2", tuple(ces))
        if key not in _prog_cache:
            _prog_cache[key] = _build_v2(ces)
        nc = _prog_cache[key]

        last_results = _run_with_retry(
            run_bass_kernel_spmd, nc, in_maps, tmpdir
        )

        inv = [0] * E
        for s, p in enumerate(perm):
            inv[p] = s
        out = np.empty((N, D), np.float32)
        pos = 0
        for e in range(E):
            s = inv[e]  # slot holding expert e
            cnt = counts[e]
            acc = np.zeros((P, MT, ces[s]), np.float32)
            for kcore in range(E):
                acc += last_results.results[kcore][f"y{s}"].reshape(
                    P, MT, ces[s]
                )
            # acc[p, m, t] -> Y^T[(m p), t] -> rows
            ye = acc.transpose(1, 0, 2).reshape(D, ces[s]).T[:cnt]
            out[pos:pos + cnt] = ye + b2[e]
            pos += cnt
        return out.reshape(B, S, D)

    # ---- v1: expert-parallel, core e owns expert e ----
    C = max(counts)
    C = (C + 7) // 8 * 8  # small alignment pad

    in_maps = []
    for e in range(E):
        cnt = counts[e]
        xp_ = np.zeros((C, D), np.float32)
        xp_[:cnt] = x_flat[idx_per_e[e]]
        # xt[p, i*C + t] = x[t, i*128 + p]
        xt = np.ascontiguousarray(
            xp_.T.reshape(DT, P, C).transpose(1, 0, 2).reshape(P, DT * C)
        ).astype(np_cdt)
        # w1[jb, p, (jj, i, c)] = W1[e][i*128+p, (jb*4+jj)*128+c]
        w1 = np.ascontiguousarray(
            W1[e].reshape(DT, P, DT, 4, P).transpose(2, 1, 3, 0, 4)
            .reshape(DT, P, 4 * DT * P)
        ).astype(np_cdt)
        # w2[m, p, (j, c)] = W2[e][j*128+p, m*128+c]
        w2 = np.ascontiguousarray(
            W2[e].reshape(FT, P, MT, P).transpose(2, 1, 0, 3)
            .reshape(MT, P, FT * P)
        ).astype(np_cdt)
        b1p = np.ascontiguousarray(b1[e].reshape(FT, P).T)
        b2p = np.ascontiguousarray(b2[e].reshape(MT, P).T)
        in_maps.append({"xt": xt, "w1": w1, "w2": w2, "b1": b1p, "b2": b2p})

    key = (C, compute_dt)
    if key not in _prog_cache:
        _prog_cache[key] = _build(C, compute_dt)
    nc = _prog_cache[key]

    last_results = _run_with_retry(
        run_bass_kernel_spmd, nc, in_maps, tmpdir
    )

    # ---- gather: grouped-by-expert concat is exactly the reference order ----
    out = np.empty((N, D), np.float32)
    pos = 0
    for e in range(E):
        cnt = counts[e]
        yt = last_results.results[e]["yt"]  # [MT, P, C] == Y^T [1024, C]
        out[pos:pos + cnt] = yt.reshape(D, C).T[:cnt]
        pos += cnt
    return out.reshape(B, S, D)

